# revision 3
# baseline (speedup 1.0000x reference)
"""DeepSeek MoE gate routing kernel for Trainium2 (Bass/Tile), 8-core SPMD.

Problem: hidden_states [4, 4096, 4096] f32, gate weight [256, 4096] f32.
  logits = x @ W^T          (T=16384 tokens, E=256 experts, h=4096)
  scores = softmax(logits)
  topk_w, topk_i = top_k(scores, 8); topk_w = topk_w / sum(topk_w) * 2.5

Sharding: tokens split across 8 cores (2048 each); W replicated.

Per-core pipeline (16 tiles of 128 tokens), fp16 compute, software-pipelined
emission: DMA runs 3 tiles ahead, cast/transpose 1 tile ahead of the matmul
so no engine FIFO head-blocks another stage.
  - DMA x tile [128, 4096] f32 (two 8KB-per-partition halves, one per ring)
  - cast f32 -> fp16: GPSIMD (3 quarters) + DVE (1 quarter); tiles 0-1 use
    DVE/ACT while the one-time GPSIMD ucode IRAM load (~6us) completes
  - PE-transpose fp16 chunks [128t,128k] -> [128k,128t], 8 chunks per PSUM
    bank [128, 1024] fp16 (1 cyc/row), PSUM->SBUF copies split DVE/ACT
  - matmul accumulate logits [128, 256] f32 over 32 k-chunks
    (stationary = x^T chunk fp16 w/ fast-weight-load, moving = W^T fp16)
  - W^T fp16 built once (DVE/ACT cast + 64 PE transposes)
  - top-8: nc.vector.max (InstMax, descending top-8) + max_index
  - weights: exp(top8 - max) on ACT, sum/reciprocal/scale on DVE
  - outputs DMA'd on the sync ring

fp16 quantization of x,W shifts ~0.3% of top-8 indices at near-ties
(weight l2 err ~3e-4), same regime as the f32r baseline (~0.16%).
"""

import numpy as np

import concourse.bass as bass
import concourse.mybir as mybir
from concourse import bacc
from concourse.bass_utils import run_bass_kernel_spmd
from concourse.masks import make_identity
from concourse.tile import TileContext

N_CORES = 8
H = 4096            # hidden size
E = 256             # n experts
TOPK = 8
T_FULL = 4 * 4096   # 16384 tokens
T_CORE = T_FULL // N_CORES  # 2048
P = 128             # partitions
N_TILES = T_CORE // P       # 16
KCH = H // P                # 32 contraction chunks
NB = 4              # transpose batches per tile (8 chunks each)
BCH = KCH // NB     # 8 chunks per batch -> [128, 1024] fp16 = one PSUM bank
Q = H // 4          # 1024-column cast quarters
SCALE = 2.5         # routed_scaling_factor

F32 = mybir.dt.float32
F16 = mybir.dt.float16


def build_bass():
    nc = bacc.Bacc(trn_type="TRN2")
    x = nc.dram_tensor("x", [T_CORE, H], F32, kind="ExternalInput")
    w = nc.dram_tensor("w", [E, H], F32, kind="ExternalInput")
    oid = nc.dram_tensor("oid", [T_CORE, TOPK], mybir.dt.int32, kind="ExternalOutput")
    owt = nc.dram_tensor("owt", [T_CORE, TOPK], F32, kind="ExternalOutput")

    with TileContext(nc) as tc:
        with (
            tc.tile_pool(name="const", bufs=1) as const_pool,
            tc.tile_pool(name="w16", bufs=1) as w16_pool,
            tc.tile_pool(name="wt", bufs=1) as wt_pool,
            tc.tile_pool(name="xin", bufs=6) as x_pool,
            tc.tile_pool(name="x16", bufs=3) as x16_pool,
            tc.tile_pool(name="xt", bufs=12) as xt_pool,
            tc.tile_pool(name="pt", bufs=4, space="PSUM") as pt_pool,
            tc.tile_pool(name="pl", bufs=3, space="PSUM") as pl_pool,
            tc.tile_pool(name="small", bufs=2) as small_pool,
        ):
            ident = const_pool.tile([P, P], F16, tag="ident")
            make_identity(nc, ident)
            # warm the GPSIMD tensor_copy ucode (one-time ~6us IRAM load)
            # during the initial DMA wait so it doesn't delay tile casts.
            gps_warm = const_pool.tile([P, 8], F16, tag="gpswarm")
            nc.gpsimd.tensor_copy(gps_warm, ident[:, 0:8])

            # ---- DMA issue helpers (x halves: one per HWDGE ring) ----
            def dma_x_tile(t):
                xin = x_pool.tile([P, H], F32, tag="xin", name=f"xin_{t}")
                nc.sync.dma_start(
                    out=xin[:, : H // 2], in_=x[t * P:(t + 1) * P, : H // 2]
                )
                nc.scalar.dma_start(
                    out=xin[:, H // 2:], in_=x[t * P:(t + 1) * P, H // 2:]
                )
                return xin

            # x tiles 0-2 + W up front so both rings stream from the start.
            xins = {0: dma_x_tile(0)}
            w0 = x_pool.tile([P, H], F32, tag="xin", name="w0")
            w1 = x_pool.tile([P, H], F32, tag="xin", name="w1")
            nc.sync.dma_start(out=w0[:, : H // 2], in_=w[0:P, : H // 2])
            nc.scalar.dma_start(out=w0[:, H // 2:], in_=w[0:P, H // 2:])
            nc.sync.dma_start(out=w1[:, : H // 2], in_=w[P:E, : H // 2])
            nc.scalar.dma_start(out=w1[:, H // 2:], in_=w[P:E, H // 2:])
            w_nat = (w0, w1)
            xins[1] = dma_x_tile(1)
            xins[2] = dma_x_tile(2)

            def cast_tile(xin, t):
                """f32 -> fp16 cast, quarter granularity.  Steady state:
                GPSIMD takes 3 quarters, DVE 1.  Tiles 0-1 avoid GPSIMD
                (ucode IRAM may still be loading) and use DVE/ACT."""
                x16 = x16_pool.tile([P, H], F16, tag="x16", name=f"x16_{t}")
                for q in range(4):
                    src = xin[:, q * Q:(q + 1) * Q]
                    dst = x16[:, q * Q:(q + 1) * Q]
                    if t >= 2:
                        eng = nc.vector if q == 1 else nc.gpsimd
                    else:
                        eng = nc.scalar if q == 3 else nc.vector
                    if eng is nc.scalar:
                        nc.scalar.copy(dst, src)
                    else:
                        eng.tensor_copy(dst, src)
                return x16

            def transpose_batch(src16, b, copy_on_vector, name):
                """PE-transpose fp16 chunks 8b..8b+7 of src16 into one PSUM
                bank [128, 1024] fp16, copy to SBUF; returns the xT tile."""
                pt = pt_pool.tile([P, BCH * P], F16, tag="pt")
                for i in range(BCH):
                    c = BCH * b + i
                    nc.tensor.matmul(
                        pt[:, i * P:(i + 1) * P],
                        lhsT=src16[:, c * P:(c + 1) * P],
                        rhs=ident,
                        is_transpose=True,
                        start=(i == 0),
                        stop=(i == BCH - 1),
                    )
                xT = xt_pool.tile([P, BCH * P], F16, tag="xt", name=name)
                if copy_on_vector:
                    nc.vector.tensor_copy(xT, pt)
                else:
                    nc.scalar.copy(xT, pt)
                return xT

            def stage_xform(t):
                x16 = cast_tile(xins.pop(t), t)
                return [
                    transpose_batch(x16, b, b % 2 == 0, f"xT{t}_{b}")
                    for b in range(NB)
                ]

            # tile-0 cast + transposes first: they only need x0 while the
            # W^T build below waits on the weight load.
            xTs = {0: stage_xform(0)}

            # ---- one-time: cast W to fp16 (DVE/ACT halves) and build
            # W^T [h, e] fp16 as 32 chunks [128, 256].  e-major order:
            # expert-half-0 batches first (w0 lands before w1).
            w16 = []
            for e in range(2):
                wt16 = w16_pool.tile([P, H], F16, tag=f"w16_{e}")
                nc.vector.tensor_copy(wt16[:, : H // 2], w_nat[e][:, : H // 2])
                nc.scalar.copy(wt16[:, H // 2:], w_nat[e][:, H // 2:])
                w16.append(wt16)

            wT = wt_pool.tile([P, KCH * E], F16, tag="wt")
            wT_r = wT.rearrange("p (c eh) -> p c eh", eh=E)
            for e in range(2):
                for b in range(NB):  # 4 batches of 8 chunks each
                    pt = pt_pool.tile([P, BCH * P], F16, tag="pt")
                    for i in range(BCH):
                        c = BCH * b + i
                        nc.tensor.matmul(
                            pt[:, i * P:(i + 1) * P],
                            lhsT=w16[e][:, c * P:(c + 1) * P],
                            rhs=ident,
                            is_transpose=True,
                            start=(i == 0),
                            stop=(i == BCH - 1),
                        )
                    dst = wT_r[:, BCH * b:BCH * b + BCH, e * P:(e + 1) * P]
                    src = pt.rearrange("p (c q) -> p c q", q=P)
                    if b % 2 == 0:
                        nc.vector.tensor_copy(dst, src)
                    else:
                        nc.scalar.copy(dst, src)

            def stage_compute(t):
                logits_ps = pl_pool.tile([P, E], F32, tag="logits")
                for b in range(NB):
                    xT = xTs[t][b]
                    for i in range(BCH):
                        c = BCH * b + i
                        nc.tensor.matmul(
                            logits_ps,
                            lhsT=xT[:, i * P:(i + 1) * P],
                            rhs=wT[:, c * E:(c + 1) * E],
                            start=(c == 0),
                            stop=(c == KCH - 1),
                        )
                del xTs[t]

                # top-8 + softmax-normalized weights (straight off PSUM)
                mx = small_pool.tile([P, TOPK], F32, tag="mx")
                nc.vector.max(out=mx, in_=logits_ps)
                idx = small_pool.tile([P, TOPK], mybir.dt.uint32, tag="idx")
                nc.vector.max_index(out=idx, in_max=mx, in_values=logits_ps)
                negm = small_pool.tile([P, 1], F32, tag="negm")
                nc.vector.tensor_scalar_mul(negm, mx[:, 0:1], -1.0)
                e8 = small_pool.tile([P, TOPK], F32, tag="e8")
                nc.scalar.activation(
                    e8, mx, mybir.ActivationFunctionType.Exp, bias=negm, scale=1.0
                )
                s8 = small_pool.tile([P, 1], F32, tag="s8")
                nc.vector.reduce_sum(s8, e8, axis=mybir.AxisListType.X)
                rcp = small_pool.tile([P, 1], F32, tag="rcp")
                nc.vector.reciprocal(rcp, s8)
                wt8 = small_pool.tile([P, TOPK], F32, tag="wt8")
                nc.vector.tensor_scalar(
                    wt8, e8, scalar1=rcp, scalar2=SCALE,
                    op0=mybir.AluOpType.mult, op1=mybir.AluOpType.mult,
                )
                nc.sync.dma_start(
                    out=oid[t * P:(t + 1) * P, :], in_=idx.bitcast(mybir.dt.int32)
                )
                nc.sync.dma_start(out=owt[t * P:(t + 1) * P, :], in_=wt8)

            # ---- software-pipelined main loop ----
            for t in range(N_TILES):
                if t + 3 < N_TILES:
                    xins[t + 3] = dma_x_tile(t + 3)
                if t + 1 < N_TILES:
                    xTs[t + 1] = stage_xform(t + 1)
                stage_compute(t)
    nc.compile()
    return nc


_NC_CACHE = {}


def _get_nc():
    if "nc" not in _NC_CACHE:
        _NC_CACHE["nc"] = build_bass()
    return _NC_CACHE["nc"]


def _ensure_ntff_hook():
    """This image's antenv lacks axon_hooks; shim it with the boot's own
    ctypes NTFF hook so trace=True works (only used by our test harness)."""
    import sys
    import types
    try:
        import antenv.axon_hooks  # noqa: F401
        return
    except ImportError:
        pass
    try:
        from trn_agent_boot.trn_boot import _ntff_profile_via_ctypes
        hook = _ntff_profile_via_ctypes("/opt/axon/libaxon_pjrt.so")
    except Exception:
        hook = None
    mod = types.ModuleType("antenv.axon_hooks")
    mod.get_axon_ntff_profile_hook = lambda: hook
    mod.set_axon_ntff_profile_hook = lambda h: None
    sys.modules["antenv.axon_hooks"] = mod
    import antenv
    antenv.axon_hooks = mod


def run(hidden_states, weight, trace=False):
    """Run on 8 NeuronCores; returns (topk_idx int32 [T,8], topk_w f32 [T,8], results)."""
    if trace:
        _ensure_ntff_hook()
    x = np.ascontiguousarray(
        np.asarray(hidden_states, dtype=np.float32).reshape(-1, H)
    )
    w = np.ascontiguousarray(np.asarray(weight, dtype=np.float32))
    assert x.shape == (T_FULL, H) and w.shape == (E, H)
    nc = _get_nc()
    in_maps = [
        {"x": np.ascontiguousarray(x[i * T_CORE:(i + 1) * T_CORE]), "w": w}
        for i in range(N_CORES)
    ]
    res = run_bass_kernel_spmd(
        nc, in_maps, core_ids=list(range(N_CORES)), trace=trace
    )
    idx = np.concatenate([r["oid"] for r in res.results], axis=0).astype(np.int32)
    wts = np.concatenate([r["owt"] for r in res.results], axis=0).astype(np.float32)
    return idx, wts, res


def kernel(hidden_states, weight):
    idx, wts, _ = run(hidden_states, weight)
    return idx, wts


# revision 6
# speedup vs baseline: 1.0819x; 1.0819x over previous
"""DeepSeek MoE gate routing kernel for Trainium2 (Bass/Tile), 8-core SPMD.

Problem: hidden_states [4, 4096, 4096] f32, gate weight [256, 4096] f32.
  logits = x @ W^T          (T=16384 tokens, E=256 experts, h=4096)
  scores = softmax(logits)
  topk_w, topk_i = top_k(scores, 8); topk_w = topk_w / sum(topk_w) * 2.5

Sharding: tokens split across 8 cores (2048 each); W replicated.

Per-core pipeline (16 tiles of 128 tokens), fp16 compute, software-pipelined
emission: DMA runs 3 tiles ahead, cast/transpose 1 tile ahead of the matmul
so no engine FIFO head-blocks another stage.
  - DMA x tile [128, 4096] f32 (one transfer per tile, rings alternating)
  - cast f32 -> fp16 on DVE/ACT (quarter granularity; GPSIMD measured 4x
    too slow at ~2.4 cyc/elem for casts)
  - PE-transpose fp16 chunks [128t,128k] -> [128k,128t], 8 chunks per PSUM
    bank [128, 1024] fp16 (1 cyc/row), PSUM->SBUF copies split DVE/ACT
  - matmul accumulate logits [128, 256] f32 over 32 k-chunks
    (stationary = x^T chunk fp16 w/ fast-weight-load, moving = W^T fp16)
  - W^T fp16 built once (DVE/ACT cast + 64 PE transposes)
  - top-8: nc.vector.max (InstMax, descending top-8) + max_index
  - weights: exp(top8 - max) on ACT, sum/reciprocal/scale on DVE
  - outputs DMA'd on the sync ring

fp16 quantization of x,W shifts ~0.3% of top-8 indices at near-ties
(weight l2 err ~3e-4), same regime as the f32r baseline (~0.16%).
"""

import numpy as np

import concourse.bass as bass
import concourse.mybir as mybir
from concourse import bacc
from concourse.bass_utils import run_bass_kernel_spmd
from concourse.masks import make_identity
from concourse.tile import TileContext

N_CORES = 8
H = 4096            # hidden size
E = 256             # n experts
TOPK = 8
T_FULL = 4 * 4096   # 16384 tokens
T_CORE = T_FULL // N_CORES  # 2048
P = 128             # partitions
N_TILES = T_CORE // P       # 16
KCH = H // P                # 32 contraction chunks
NB = 4              # transpose batches per tile (8 chunks each)
BCH = KCH // NB     # 8 chunks per batch -> [128, 1024] fp16 = one PSUM bank
Q = H // 4          # 1024-column cast quarters
SCALE = 2.5         # routed_scaling_factor

F32 = mybir.dt.float32
F16 = mybir.dt.float16


def build_bass():
    nc = bacc.Bacc(trn_type="TRN2")
    x = nc.dram_tensor("x", [T_CORE, H], F32, kind="ExternalInput")
    w = nc.dram_tensor("w", [E, H], F32, kind="ExternalInput")
    oid = nc.dram_tensor("oid", [T_CORE, TOPK], mybir.dt.int32, kind="ExternalOutput")
    owt = nc.dram_tensor("owt", [T_CORE, TOPK], F32, kind="ExternalOutput")

    with TileContext(nc) as tc:
        with (
            tc.tile_pool(name="const", bufs=1) as const_pool,
            tc.tile_pool(name="w16", bufs=1) as w16_pool,
            tc.tile_pool(name="wt", bufs=1) as wt_pool,
            tc.tile_pool(name="xin", bufs=6) as x_pool,
            tc.tile_pool(name="x16", bufs=3) as x16_pool,
            tc.tile_pool(name="xt", bufs=12) as xt_pool,
            tc.tile_pool(name="pt", bufs=4, space="PSUM") as pt_pool,
            tc.tile_pool(name="pl", bufs=3, space="PSUM") as pl_pool,
            tc.tile_pool(name="small", bufs=2) as small_pool,
        ):
            ident = const_pool.tile([P, P], F16, tag="ident")
            make_identity(nc, ident)

            # ---- DMA issue: one full-tile transfer, rings alternating by
            # tile (128 descriptors of 16KB each, spread over 16 engines) ----
            def dma_x_tile(t):
                xin = x_pool.tile([P, H], F32, tag="xin", name=f"xin_{t}")
                eng = nc.sync if t % 2 == 0 else nc.scalar
                eng.dma_start(out=xin, in_=x[t * P:(t + 1) * P, :])
                return xin

            # x tiles 0-2 + W up front so both rings stream from the start.
            xins = {0: dma_x_tile(0)}
            w0 = x_pool.tile([P, H], F32, tag="xin", name="w0")
            w1 = x_pool.tile([P, H], F32, tag="xin", name="w1")
            nc.sync.dma_start(out=w0[:, : H // 2], in_=w[0:P, : H // 2])
            nc.scalar.dma_start(out=w0[:, H // 2:], in_=w[0:P, H // 2:])
            nc.sync.dma_start(out=w1[:, : H // 2], in_=w[P:E, : H // 2])
            nc.scalar.dma_start(out=w1[:, H // 2:], in_=w[P:E, H // 2:])
            w_nat = (w0, w1)
            xins[1] = dma_x_tile(1)
            xins[2] = dma_x_tile(2)

            def cast_tile(xin, t):
                """f32 -> fp16 cast, quarter granularity, DVE/ACT split."""
                x16 = x16_pool.tile([P, H], F16, tag="x16", name=f"x16_{t}")
                for q in range(4):
                    src = xin[:, q * Q:(q + 1) * Q]
                    dst = x16[:, q * Q:(q + 1) * Q]
                    if q % 2 == 0:
                        nc.vector.tensor_copy(dst, src)
                    else:
                        nc.scalar.copy(dst, src)
                return x16

            def transpose_batch(src16, b, copy_on_vector, name):
                """PE-transpose fp16 chunks 8b..8b+7 of src16 into one PSUM
                bank [128, 1024] fp16, copy to SBUF; returns the xT tile."""
                pt = pt_pool.tile([P, BCH * P], F16, tag="pt")
                for i in range(BCH):
                    c = BCH * b + i
                    nc.tensor.matmul(
                        pt[:, i * P:(i + 1) * P],
                        lhsT=src16[:, c * P:(c + 1) * P],
                        rhs=ident,
                        is_transpose=True,
                        start=(i == 0),
                        stop=(i == BCH - 1),
                    )
                xT = xt_pool.tile([P, BCH * P], F16, tag="xt", name=name)
                if copy_on_vector:
                    nc.vector.tensor_copy(xT, pt)
                else:
                    nc.scalar.copy(xT, pt)
                return xT

            def stage_xform(t):
                x16 = cast_tile(xins.pop(t), t)
                return [
                    transpose_batch(x16, b, b % 2 == 0, f"xT{t}_{b}")
                    for b in range(NB)
                ]

            # tile-0 cast + transposes first: they only need x0 while the
            # W^T build below waits on the weight load.
            xTs = {0: stage_xform(0)}

            # ---- one-time: cast W to fp16 (DVE/ACT halves) and build
            # W^T [h, e] fp16 as 32 chunks [128, 256].  e-major order:
            # expert-half-0 batches first (w0 lands before w1).
            w16 = []
            for e in range(2):
                wt16 = w16_pool.tile([P, H], F16, tag=f"w16_{e}")
                nc.vector.tensor_copy(wt16[:, : H // 2], w_nat[e][:, : H // 2])
                nc.scalar.copy(wt16[:, H // 2:], w_nat[e][:, H // 2:])
                w16.append(wt16)

            wT = wt_pool.tile([P, KCH * E], F16, tag="wt")
            wT_r = wT.rearrange("p (c eh) -> p c eh", eh=E)
            for e in range(2):
                for b in range(NB):  # 4 batches of 8 chunks each
                    pt = pt_pool.tile([P, BCH * P], F16, tag="pt")
                    for i in range(BCH):
                        c = BCH * b + i
                        nc.tensor.matmul(
                            pt[:, i * P:(i + 1) * P],
                            lhsT=w16[e][:, c * P:(c + 1) * P],
                            rhs=ident,
                            is_transpose=True,
                            start=(i == 0),
                            stop=(i == BCH - 1),
                        )
                    dst = wT_r[:, BCH * b:BCH * b + BCH, e * P:(e + 1) * P]
                    src = pt.rearrange("p (c q) -> p c q", q=P)
                    if b % 2 == 0:
                        nc.vector.tensor_copy(dst, src)
                    else:
                        nc.scalar.copy(dst, src)

            def stage_compute(t):
                logits_ps = pl_pool.tile([P, E], F32, tag="logits")
                for b in range(NB):
                    xT = xTs[t][b]
                    for i in range(BCH):
                        c = BCH * b + i
                        nc.tensor.matmul(
                            logits_ps,
                            lhsT=xT[:, i * P:(i + 1) * P],
                            rhs=wT[:, c * E:(c + 1) * E],
                            start=(c == 0),
                            stop=(c == KCH - 1),
                        )
                del xTs[t]

                # top-8 + softmax-normalized weights (straight off PSUM)
                mx = small_pool.tile([P, TOPK], F32, tag="mx")
                nc.vector.max(out=mx, in_=logits_ps)
                idx = small_pool.tile([P, TOPK], mybir.dt.uint32, tag="idx")
                nc.vector.max_index(out=idx, in_max=mx, in_values=logits_ps)
                negm = small_pool.tile([P, 1], F32, tag="negm")
                nc.vector.tensor_scalar_mul(negm, mx[:, 0:1], -1.0)
                e8 = small_pool.tile([P, TOPK], F32, tag="e8")
                nc.scalar.activation(
                    e8, mx, mybir.ActivationFunctionType.Exp, bias=negm, scale=1.0
                )
                s8 = small_pool.tile([P, 1], F32, tag="s8")
                nc.vector.reduce_sum(s8, e8, axis=mybir.AxisListType.X)
                rcp = small_pool.tile([P, 1], F32, tag="rcp")
                nc.vector.reciprocal(rcp, s8)
                wt8 = small_pool.tile([P, TOPK], F32, tag="wt8")
                nc.vector.tensor_scalar(
                    wt8, e8, scalar1=rcp, scalar2=SCALE,
                    op0=mybir.AluOpType.mult, op1=mybir.AluOpType.mult,
                )
                nc.sync.dma_start(
                    out=oid[t * P:(t + 1) * P, :], in_=idx.bitcast(mybir.dt.int32)
                )
                nc.sync.dma_start(out=owt[t * P:(t + 1) * P, :], in_=wt8)

            # ---- software-pipelined main loop ----
            for t in range(N_TILES):
                if t + 3 < N_TILES:
                    xins[t + 3] = dma_x_tile(t + 3)
                if t + 1 < N_TILES:
                    xTs[t + 1] = stage_xform(t + 1)
                stage_compute(t)
    nc.compile()
    return nc


_NC_CACHE = {}


def _get_nc():
    if "nc" not in _NC_CACHE:
        _NC_CACHE["nc"] = build_bass()
    return _NC_CACHE["nc"]


def _ensure_ntff_hook():
    """This image's antenv lacks axon_hooks; shim it with the boot's own
    ctypes NTFF hook so trace=True works (only used by our test harness)."""
    import sys
    import types
    try:
        import antenv.axon_hooks  # noqa: F401
        return
    except ImportError:
        pass
    try:
        from trn_agent_boot.trn_boot import _ntff_profile_via_ctypes
        hook = _ntff_profile_via_ctypes("/opt/axon/libaxon_pjrt.so")
    except Exception:
        hook = None
    mod = types.ModuleType("antenv.axon_hooks")
    mod.get_axon_ntff_profile_hook = lambda: hook
    mod.set_axon_ntff_profile_hook = lambda h: None
    sys.modules["antenv.axon_hooks"] = mod
    import antenv
    antenv.axon_hooks = mod


def run(hidden_states, weight, trace=False):
    """Run on 8 NeuronCores; returns (topk_idx int32 [T,8], topk_w f32 [T,8], results)."""
    if trace:
        _ensure_ntff_hook()
    x = np.ascontiguousarray(
        np.asarray(hidden_states, dtype=np.float32).reshape(-1, H)
    )
    w = np.ascontiguousarray(np.asarray(weight, dtype=np.float32))
    assert x.shape == (T_FULL, H) and w.shape == (E, H)
    nc = _get_nc()
    in_maps = [
        {"x": np.ascontiguousarray(x[i * T_CORE:(i + 1) * T_CORE]), "w": w}
        for i in range(N_CORES)
    ]
    res = run_bass_kernel_spmd(
        nc, in_maps, core_ids=list(range(N_CORES)), trace=trace
    )
    idx = np.concatenate([r["oid"] for r in res.results], axis=0).astype(np.int32)
    wts = np.concatenate([r["owt"] for r in res.results], axis=0).astype(np.float32)
    return idx, wts, res


def kernel(hidden_states, weight):
    idx, wts, _ = run(hidden_states, weight)
    return idx, wts


# revision 10
# speedup vs baseline: 1.2613x; 1.1658x over previous
"""DeepSeek MoE gate routing kernel for Trainium2 (Bass/Tile), 8-core SPMD.

Problem: hidden_states [4, 4096, 4096] f32, gate weight [256, 4096] f32.
  logits = x @ W^T          (T=16384 tokens, E=256 experts, h=4096)
  scores = softmax(logits)
  topk_w, topk_i = top_k(scores, 8); topk_w = topk_w / sum(topk_w) * 2.5

Sharding: tokens split across 8 cores (2048 each); W replicated.

Per-core pipeline (16 tiles of 128 tokens), fp16 compute, software-pipelined
emission: DMA runs 3 tiles ahead, cast/transpose 1 tile ahead of the matmul
so no engine FIFO head-blocks another stage.
  - DMA x tile [128, 4096] f32 (one transfer per tile, rings alternating)
  - cast f32 -> fp16 on DVE/ACT (quarter granularity; GPSIMD measured 4x
    too slow at ~2.4 cyc/elem for casts)
  - PE-transpose fp16 chunks [128t,128k] -> [128k,128t], 8 chunks per PSUM
    bank [128, 1024] fp16 (1 cyc/row), PSUM->SBUF copies split DVE/ACT
  - matmul accumulate logits [128, 256] f32 over 32 k-chunks
    (stationary = x^T chunk fp16 w/ fast-weight-load, moving = W^T fp16)
  - W^T fp16 built once (DVE/ACT cast + 64 PE transposes)
  - top-8: nc.vector.max (InstMax, descending top-8) + max_index
  - weights: exp(top8 - max) on ACT, sum/reciprocal/scale on DVE
  - outputs DMA'd on the sync ring

fp16 quantization of x,W shifts ~0.3% of top-8 indices at near-ties
(weight l2 err ~3e-4), same regime as the f32r baseline (~0.16%).
"""

import numpy as np

import concourse.bass as bass
import concourse.mybir as mybir
from concourse import bacc
from concourse.bass_utils import run_bass_kernel_spmd
from concourse.masks import make_identity
from concourse.tile import TileContext

N_CORES = 8
H = 4096            # hidden size
E = 256             # n experts
TOPK = 8
T_FULL = 4 * 4096   # 16384 tokens
T_CORE = T_FULL // N_CORES  # 2048
P = 128             # partitions
N_TILES = T_CORE // P       # 16
KCH = H // P                # 32 contraction chunks
NB = 4              # transpose batches per tile (8 chunks each)
BCH = KCH // NB     # 8 chunks per batch -> [128, 1024] fp16 = one PSUM bank
Q = H // 4          # 1024-column cast quarters
SCALE = 2.5         # routed_scaling_factor

F32 = mybir.dt.float32
F16 = mybir.dt.float16


def build_bass():
    nc = bacc.Bacc(trn_type="TRN2")
    x = nc.dram_tensor("x", [T_CORE, H], F32, kind="ExternalInput")
    w = nc.dram_tensor("w", [E, H], F32, kind="ExternalInput")
    oid = nc.dram_tensor("oid", [T_CORE, TOPK], mybir.dt.int32, kind="ExternalOutput")
    owt = nc.dram_tensor("owt", [T_CORE, TOPK], F32, kind="ExternalOutput")

    with TileContext(nc) as tc:
        with (
            tc.tile_pool(name="const", bufs=1) as const_pool,
            tc.tile_pool(name="w16", bufs=1) as w16_pool,
            tc.tile_pool(name="wt", bufs=1) as wt_pool,
            tc.tile_pool(name="xin", bufs=6) as x_pool,
            tc.tile_pool(name="x16", bufs=3) as x16_pool,
            tc.tile_pool(name="xt", bufs=12) as xt_pool,
            tc.tile_pool(name="pt", bufs=4, space="PSUM") as pt_pool,
            tc.tile_pool(name="pl", bufs=3, space="PSUM") as pl_pool,
            tc.tile_pool(name="small", bufs=2) as small_pool,
        ):
            ident = const_pool.tile([P, P], F16, tag="ident")
            make_identity(nc, ident)

            # ---- DMA issue: half-tile per HWDGE ring so the first cast
            # quarters can start before the whole tile lands ----
            def dma_x_tile(t):
                xin = x_pool.tile([P, H], F32, tag="xin", name=f"xin_{t}")
                nc.sync.dma_start(
                    out=xin[:, : H // 2], in_=x[t * P:(t + 1) * P, : H // 2]
                )
                nc.scalar.dma_start(
                    out=xin[:, H // 2:], in_=x[t * P:(t + 1) * P, H // 2:]
                )
                return xin

            # x tiles 0-2 + W up front so both rings stream from the start.
            xins = {0: dma_x_tile(0)}
            w0 = x_pool.tile([P, H], F32, tag="xin", name="w0")
            w1 = x_pool.tile([P, H], F32, tag="xin", name="w1")
            nc.sync.dma_start(out=w0[:, : H // 2], in_=w[0:P, : H // 2])
            nc.scalar.dma_start(out=w0[:, H // 2:], in_=w[0:P, H // 2:])
            nc.sync.dma_start(out=w1[:, : H // 2], in_=w[P:E, : H // 2])
            nc.scalar.dma_start(out=w1[:, H // 2:], in_=w[P:E, H // 2:])
            w_nat = (w0, w1)
            xins[1] = dma_x_tile(1)
            xins[2] = dma_x_tile(2)

            def cast_tile(xin, t):
                """f32 -> fp16 cast, quarter granularity, DVE/ACT split."""
                x16 = x16_pool.tile([P, H], F16, tag="x16", name=f"x16_{t}")
                for q in range(4):
                    src = xin[:, q * Q:(q + 1) * Q]
                    dst = x16[:, q * Q:(q + 1) * Q]
                    if q % 2 == 0:
                        nc.vector.tensor_copy(dst, src)
                    else:
                        nc.scalar.copy(dst, src)
                return x16

            def transpose_batch(src16, b, copy_on_vector, name):
                """PE-transpose fp16 chunks 8b..8b+7 of src16 into one PSUM
                bank [128, 1024] fp16, copy to SBUF; returns the xT tile."""
                pt = pt_pool.tile([P, BCH * P], F16, tag="pt")
                for i in range(BCH):
                    c = BCH * b + i
                    nc.tensor.matmul(
                        pt[:, i * P:(i + 1) * P],
                        lhsT=src16[:, c * P:(c + 1) * P],
                        rhs=ident,
                        is_transpose=True,
                        start=(i == 0),
                        stop=(i == BCH - 1),
                    )
                xT = xt_pool.tile([P, BCH * P], F16, tag="xt", name=name)
                if copy_on_vector:
                    nc.vector.tensor_copy(xT, pt)
                else:
                    nc.scalar.copy(xT, pt)
                return xT

            def stage_cast(t):
                return cast_tile(xins.pop(t), t)

            def stage_transpose(t):
                x16 = x16s.pop(t)
                return [
                    transpose_batch(x16, b, b % 2 == 0, f"xT{t}_{b}")
                    for b in range(NB)
                ]

            # tile-0/1 casts + tile-0 transposes first: they only need x
            # while the W^T build below waits on the weight load.
            x16s = {0: stage_cast(0)}
            xTs = {0: stage_transpose(0)}
            x16s[1] = stage_cast(1)

            # ---- one-time: cast W to fp16 (DVE/ACT halves) and build
            # W^T [h, e] fp16 as 32 chunks [128, 256].  e-major order:
            # expert-half-0 batches first (w0 lands before w1).
            w16 = []
            for e in range(2):
                wt16 = w16_pool.tile([P, H], F16, tag=f"w16_{e}")
                nc.vector.tensor_copy(wt16[:, : H // 2], w_nat[e][:, : H // 2])
                nc.scalar.copy(wt16[:, H // 2:], w_nat[e][:, H // 2:])
                w16.append(wt16)

            wT = wt_pool.tile([P, KCH * E], F16, tag="wt")
            wT_r = wT.rearrange("p (c eh) -> p c eh", eh=E)
            for e in range(2):
                for b in range(NB):  # 4 batches of 8 chunks each
                    pt = pt_pool.tile([P, BCH * P], F16, tag="pt")
                    for i in range(BCH):
                        c = BCH * b + i
                        nc.tensor.matmul(
                            pt[:, i * P:(i + 1) * P],
                            lhsT=w16[e][:, c * P:(c + 1) * P],
                            rhs=ident,
                            is_transpose=True,
                            start=(i == 0),
                            stop=(i == BCH - 1),
                        )
                    dst = wT_r[:, BCH * b:BCH * b + BCH, e * P:(e + 1) * P]
                    src = pt.rearrange("p (c q) -> p c q", q=P)
                    if b % 2 == 0:
                        nc.vector.tensor_copy(dst, src)
                    else:
                        nc.scalar.copy(dst, src)

            def stage_compute(t):
                logits_ps = pl_pool.tile([P, E], F32, tag="logits")
                for b in range(NB):
                    xT = xTs[t][b]
                    for i in range(BCH):
                        c = BCH * b + i
                        nc.tensor.matmul(
                            logits_ps,
                            lhsT=xT[:, i * P:(i + 1) * P],
                            rhs=wT[:, c * E:(c + 1) * E],
                            start=(c == 0),
                            stop=(c == KCH - 1),
                        )
                del xTs[t]

                # top-8 + softmax-normalized weights (straight off PSUM).
                # logits are O(6) so exp() cannot overflow f32 -- no max
                # subtraction needed; exp runs on ACT parallel to max_index.
                mx = small_pool.tile([P, TOPK], F32, tag="mx")
                nc.vector.max(out=mx, in_=logits_ps)
                idx = small_pool.tile([P, TOPK], mybir.dt.uint32, tag="idx")
                nc.vector.max_index(out=idx, in_max=mx, in_values=logits_ps)
                e8 = small_pool.tile([P, TOPK], F32, tag="e8")
                nc.scalar.activation(
                    e8, mx, mybir.ActivationFunctionType.Exp, scale=1.0
                )
                s8 = small_pool.tile([P, 1], F32, tag="s8")
                nc.vector.reduce_sum(s8, e8, axis=mybir.AxisListType.X)
                rcp = small_pool.tile([P, 1], F32, tag="rcp")
                nc.vector.reciprocal(rcp, s8)
                wt8 = small_pool.tile([P, TOPK], F32, tag="wt8")
                nc.vector.tensor_scalar(
                    wt8, e8, scalar1=rcp, scalar2=SCALE,
                    op0=mybir.AluOpType.mult, op1=mybir.AluOpType.mult,
                )
                nc.sync.dma_start(
                    out=oid[t * P:(t + 1) * P, :], in_=idx.bitcast(mybir.dt.int32)
                )
                nc.sync.dma_start(out=owt[t * P:(t + 1) * P, :], in_=wt8)

            # ---- software-pipelined main loop: DMA 3 ahead, cast 2 ahead,
            # transpose 1 ahead of the matmul+topk so the next tile's casts
            # never queue behind this tile's topk in the DVE/ACT FIFOs ----
            for t in range(N_TILES):
                if t + 3 < N_TILES:
                    xins[t + 3] = dma_x_tile(t + 3)
                if t + 2 < N_TILES:
                    x16s[t + 2] = stage_cast(t + 2)
                if t + 1 < N_TILES:
                    xTs[t + 1] = stage_transpose(t + 1)
                stage_compute(t)
    nc.compile()
    return nc


_NC_CACHE = {}


def _get_nc():
    if "nc" not in _NC_CACHE:
        _NC_CACHE["nc"] = build_bass()
    return _NC_CACHE["nc"]


def _ensure_ntff_hook():
    """This image's antenv lacks axon_hooks; shim it with the boot's own
    ctypes NTFF hook so trace=True works (only used by our test harness)."""
    import sys
    import types
    try:
        import antenv.axon_hooks  # noqa: F401
        return
    except ImportError:
        pass
    try:
        from trn_agent_boot.trn_boot import _ntff_profile_via_ctypes
        hook = _ntff_profile_via_ctypes("/opt/axon/libaxon_pjrt.so")
    except Exception:
        hook = None
    mod = types.ModuleType("antenv.axon_hooks")
    mod.get_axon_ntff_profile_hook = lambda: hook
    mod.set_axon_ntff_profile_hook = lambda h: None
    sys.modules["antenv.axon_hooks"] = mod
    import antenv
    antenv.axon_hooks = mod


def run(hidden_states, weight, trace=False):
    """Run on 8 NeuronCores; returns (topk_idx int32 [T,8], topk_w f32 [T,8], results)."""
    if trace:
        _ensure_ntff_hook()
    x = np.ascontiguousarray(
        np.asarray(hidden_states, dtype=np.float32).reshape(-1, H)
    )
    w = np.ascontiguousarray(np.asarray(weight, dtype=np.float32))
    assert x.shape == (T_FULL, H) and w.shape == (E, H)
    nc = _get_nc()
    in_maps = [
        {"x": np.ascontiguousarray(x[i * T_CORE:(i + 1) * T_CORE]), "w": w}
        for i in range(N_CORES)
    ]
    res = run_bass_kernel_spmd(
        nc, in_maps, core_ids=list(range(N_CORES)), trace=trace
    )
    idx = np.concatenate([r["oid"] for r in res.results], axis=0).astype(np.int32)
    wts = np.concatenate([r["owt"] for r in res.results], axis=0).astype(np.float32)
    return idx, wts, res


def kernel(hidden_states, weight):
    idx, wts, _ = run(hidden_states, weight)
    return idx, wts


# revision 14
# speedup vs baseline: 1.2650x; 1.0030x over previous
"""DeepSeek MoE gate routing kernel for Trainium2 (Bass/Tile), 8-core SPMD.

Problem: hidden_states [4, 4096, 4096] f32, gate weight [256, 4096] f32.
  logits = x @ W^T          (T=16384 tokens, E=256 experts, h=4096)
  scores = softmax(logits)
  topk_w, topk_i = top_k(scores, 8); topk_w = topk_w / sum(topk_w) * 2.5

Sharding: tokens split across 8 cores (2048 each); W replicated.

Per-core pipeline (16 tiles of 128 tokens), fp16 compute, software-pipelined
emission: DMA runs 3 tiles ahead, cast/transpose 1 tile ahead of the matmul
so no engine FIFO head-blocks another stage.
  - DMA x tile [128, 4096] f32 (one transfer per tile, rings alternating)
  - cast f32 -> fp16 on DVE/ACT (quarter granularity; GPSIMD measured 4x
    too slow at ~2.4 cyc/elem for casts)
  - PE-transpose fp16 chunks [128t,128k] -> [128k,128t], 8 chunks per PSUM
    bank [128, 1024] fp16 (1 cyc/row), PSUM->SBUF copies split DVE/ACT
  - matmul accumulate logits [128, 256] f32 over 32 k-chunks
    (stationary = x^T chunk fp16 w/ fast-weight-load, moving = W^T fp16)
  - W^T fp16 built once (DVE/ACT cast + 64 PE transposes)
  - top-8: nc.vector.max (InstMax, descending top-8) + max_index
  - weights: exp(top8 - max) on ACT, sum/reciprocal/scale on DVE
  - outputs DMA'd on the sync ring

fp16 quantization of x,W shifts ~0.3% of top-8 indices at near-ties
(weight l2 err ~3e-4), same regime as the f32r baseline (~0.16%).
"""

import numpy as np

import concourse.bass as bass
import concourse.mybir as mybir
from concourse import bacc
from concourse.bass_utils import run_bass_kernel_spmd
from concourse.masks import make_identity
from concourse.tile import TileContext

N_CORES = 8
H = 4096            # hidden size
E = 256             # n experts
TOPK = 8
T_FULL = 4 * 4096   # 16384 tokens
T_CORE = T_FULL // N_CORES  # 2048
P = 128             # partitions
N_TILES = T_CORE // P       # 16
KCH = H // P                # 32 contraction chunks
NB = 2              # transpose batches per tile (16 chunks each)
BCH = KCH // NB     # 16 chunks per batch -> [128, 2048] fp16 = two PSUM banks
BANK_CH = 8         # chunks per 2KB PSUM bank (one accumulation group each)
Q = H // 4          # 1024-column cast quarters
SCALE = 2.5         # routed_scaling_factor

F32 = mybir.dt.float32
F16 = mybir.dt.float16


def build_bass():
    nc = bacc.Bacc(trn_type="TRN2")
    x = nc.dram_tensor("x", [T_CORE, H], F32, kind="ExternalInput")
    w = nc.dram_tensor("w", [E, H], F32, kind="ExternalInput")
    oid = nc.dram_tensor("oid", [T_CORE, TOPK], mybir.dt.int32, kind="ExternalOutput")
    owt = nc.dram_tensor("owt", [T_CORE, TOPK], F32, kind="ExternalOutput")

    with TileContext(nc) as tc:
        with (
            tc.tile_pool(name="const", bufs=1) as const_pool,
            tc.tile_pool(name="w16", bufs=1) as w16_pool,
            tc.tile_pool(name="wt", bufs=1) as wt_pool,
            tc.tile_pool(name="xin", bufs=7) as x_pool,
            tc.tile_pool(name="x16", bufs=3) as x16_pool,
            tc.tile_pool(name="xt", bufs=6) as xt_pool,
            tc.tile_pool(name="pt", bufs=2, space="PSUM") as pt_pool,
            tc.tile_pool(name="pl", bufs=3, space="PSUM") as pl_pool,
            tc.tile_pool(name="small", bufs=2) as small_pool,
        ):
            ident = const_pool.tile([P, P], F16, tag="ident")
            make_identity(nc, ident)

            # ---- DMA issue: half-tile per HWDGE ring so the first cast
            # quarters can start before the whole tile lands ----
            def dma_x_tile(t):
                xin = x_pool.tile([P, H], F32, tag="xin", name=f"xin_{t}")
                nc.sync.dma_start(
                    out=xin[:, : H // 2], in_=x[t * P:(t + 1) * P, : H // 2]
                )
                nc.scalar.dma_start(
                    out=xin[:, H // 2:], in_=x[t * P:(t + 1) * P, H // 2:]
                )
                return xin

            # x tile 0 + W up front (W whole-half per ring so it lands early:
            # the W^T build gates every logits matmul), then x tiles 1-2.
            xins = {0: dma_x_tile(0)}
            w0 = x_pool.tile([P, H], F32, tag="xin", name="w0")
            w1 = x_pool.tile([P, H], F32, tag="xin", name="w1")
            nc.sync.dma_start(out=w0, in_=w[0:P, :])
            nc.scalar.dma_start(out=w1, in_=w[P:E, :])
            w_nat = (w0, w1)
            xins[1] = dma_x_tile(1)
            xins[2] = dma_x_tile(2)

            def cast_tile(xin, t):
                """f32 -> fp16 cast, quarter granularity, DVE/ACT split."""
                x16 = x16_pool.tile([P, H], F16, tag="x16", name=f"x16_{t}")
                for q in range(4):
                    src = xin[:, q * Q:(q + 1) * Q]
                    dst = x16[:, q * Q:(q + 1) * Q]
                    if q % 2 == 0:
                        nc.vector.tensor_copy(dst, src)
                    else:
                        nc.scalar.copy(dst, src)
                return x16

            def transpose_batch(src16, b, copy_on_vector, name):
                """PE-transpose fp16 chunks 16b..16b+15 of src16 into a
                two-bank PSUM tile [128, 2048] fp16 (one accumulation group
                per 2KB bank), one copy to SBUF; returns the xT tile."""
                pt = pt_pool.tile([P, BCH * P], F16, tag="pt")
                for i in range(BCH):
                    c = BCH * b + i
                    nc.tensor.matmul(
                        pt[:, i * P:(i + 1) * P],
                        lhsT=src16[:, c * P:(c + 1) * P],
                        rhs=ident,
                        is_transpose=True,
                        start=(i % BANK_CH == 0),
                        stop=(i % BANK_CH == BANK_CH - 1),
                    )
                xT = xt_pool.tile([P, BCH * P], F16, tag="xt", name=name)
                if copy_on_vector:
                    nc.vector.tensor_copy(xT, pt)
                else:
                    nc.scalar.copy(xT, pt)
                return xT

            def stage_cast(t):
                return cast_tile(xins.pop(t), t)

            def stage_transpose(t):
                x16 = x16s.pop(t)
                return [
                    transpose_batch(x16, b, b % 2 == 0, f"xT{t}_{b}")
                    for b in range(NB)
                ]

            # tile-0/1 casts + tile-0 transposes first: they only need x
            # while the W^T build below waits on the weight load.
            x16s = {0: stage_cast(0)}
            xTs = {0: stage_transpose(0)}

            # ---- one-time: cast W to fp16 (DVE/ACT halves) and build
            # W^T [h, e] fp16 as 32 chunks [128, 256].  e-major order:
            # expert-half-0 batches first (w0 lands before w1).
            w16 = []
            for e in range(2):
                wt16 = w16_pool.tile([P, H], F16, tag=f"w16_{e}")
                nc.vector.tensor_copy(wt16[:, : H // 2], w_nat[e][:, : H // 2])
                nc.scalar.copy(wt16[:, H // 2:], w_nat[e][:, H // 2:])
                w16.append(wt16)

            wT = wt_pool.tile([P, KCH * E], F16, tag="wt")
            wT_r = wT.rearrange("p (c eh) -> p c eh", eh=E)
            for e in range(2):
                for b in range(NB):  # 2 batches of 16 chunks each
                    pt = pt_pool.tile([P, BCH * P], F16, tag="pt")
                    for i in range(BCH):
                        c = BCH * b + i
                        nc.tensor.matmul(
                            pt[:, i * P:(i + 1) * P],
                            lhsT=w16[e][:, c * P:(c + 1) * P],
                            rhs=ident,
                            is_transpose=True,
                            start=(i % BANK_CH == 0),
                            stop=(i % BANK_CH == BANK_CH - 1),
                        )
                    dst = wT_r[:, BCH * b:BCH * b + BCH, e * P:(e + 1) * P]
                    srcv = pt.rearrange("p (c q) -> p c q", q=P)
                    if b % 2 == 0:
                        nc.vector.tensor_copy(dst, srcv)
                    else:
                        nc.scalar.copy(dst, srcv)

            # tile-1 cast after the W casts so W^T (which gates all logits
            # matmuls) is never stuck behind tile casts in the DVE/ACT FIFOs.
            x16s[1] = stage_cast(1)

            def stage_compute(t):
                logits_ps = pl_pool.tile([P, E], F32, tag="logits")
                for b in range(NB):
                    xT = xTs[t][b]
                    for i in range(BCH):
                        c = BCH * b + i
                        nc.tensor.matmul(
                            logits_ps,
                            lhsT=xT[:, i * P:(i + 1) * P],
                            rhs=wT[:, c * E:(c + 1) * E],
                            start=(c == 0),
                            stop=(c == KCH - 1),
                        )
                del xTs[t]

                # top-8 + softmax-normalized weights (straight off PSUM).
                # logits are O(6) so exp() cannot overflow f32 -- no max
                # subtraction needed; exp runs on ACT parallel to max_index.
                mx = small_pool.tile([P, TOPK], F32, tag="mx")
                nc.vector.max(out=mx, in_=logits_ps)
                idx = small_pool.tile([P, TOPK], mybir.dt.uint32, tag="idx")
                nc.vector.max_index(out=idx, in_max=mx, in_values=logits_ps)
                e8 = small_pool.tile([P, TOPK], F32, tag="e8")
                nc.scalar.activation(
                    e8, mx, mybir.ActivationFunctionType.Exp, scale=1.0
                )
                s8 = small_pool.tile([P, 1], F32, tag="s8")
                nc.vector.reduce_sum(s8, e8, axis=mybir.AxisListType.X)
                rcp = small_pool.tile([P, 1], F32, tag="rcp")
                nc.vector.reciprocal(rcp, s8)
                wt8 = small_pool.tile([P, TOPK], F32, tag="wt8")
                nc.vector.tensor_scalar(
                    wt8, e8, scalar1=rcp, scalar2=SCALE,
                    op0=mybir.AluOpType.mult, op1=mybir.AluOpType.mult,
                )
                nc.sync.dma_start(
                    out=oid[t * P:(t + 1) * P, :], in_=idx.bitcast(mybir.dt.int32)
                )
                nc.sync.dma_start(out=owt[t * P:(t + 1) * P, :], in_=wt8)

            # ---- software-pipelined main loop: DMA 3 ahead, cast 2 ahead,
            # transpose 1 ahead of the matmul+topk so the next tile's casts
            # never queue behind this tile's topk in the DVE/ACT FIFOs ----
            for t in range(N_TILES):
                if t + 3 < N_TILES:
                    xins[t + 3] = dma_x_tile(t + 3)
                if t + 2 < N_TILES:
                    x16s[t + 2] = stage_cast(t + 2)
                if t + 1 < N_TILES:
                    xTs[t + 1] = stage_transpose(t + 1)
                stage_compute(t)
    nc.compile()
    return nc


_NC_CACHE = {}


def _get_nc():
    if "nc" not in _NC_CACHE:
        _NC_CACHE["nc"] = build_bass()
    return _NC_CACHE["nc"]


def _ensure_ntff_hook():
    """This image's antenv lacks axon_hooks; shim it with the boot's own
    ctypes NTFF hook so trace=True works (only used by our test harness)."""
    import sys
    import types
    try:
        import antenv.axon_hooks  # noqa: F401
        return
    except ImportError:
        pass
    try:
        from trn_agent_boot.trn_boot import _ntff_profile_via_ctypes
        hook = _ntff_profile_via_ctypes("/opt/axon/libaxon_pjrt.so")
    except Exception:
        hook = None
    mod = types.ModuleType("antenv.axon_hooks")
    mod.get_axon_ntff_profile_hook = lambda: hook
    mod.set_axon_ntff_profile_hook = lambda h: None
    sys.modules["antenv.axon_hooks"] = mod
    import antenv
    antenv.axon_hooks = mod


def run(hidden_states, weight, trace=False):
    """Run on 8 NeuronCores; returns (topk_idx int32 [T,8], topk_w f32 [T,8], results)."""
    if trace:
        _ensure_ntff_hook()
    x = np.ascontiguousarray(
        np.asarray(hidden_states, dtype=np.float32).reshape(-1, H)
    )
    w = np.ascontiguousarray(np.asarray(weight, dtype=np.float32))
    assert x.shape == (T_FULL, H) and w.shape == (E, H)
    nc = _get_nc()
    in_maps = [
        {"x": np.ascontiguousarray(x[i * T_CORE:(i + 1) * T_CORE]), "w": w}
        for i in range(N_CORES)
    ]
    res = run_bass_kernel_spmd(
        nc, in_maps, core_ids=list(range(N_CORES)), trace=trace
    )
    idx = np.concatenate([r["oid"] for r in res.results], axis=0).astype(np.int32)
    wts = np.concatenate([r["owt"] for r in res.results], axis=0).astype(np.float32)
    return idx, wts, res


def kernel(hidden_states, weight):
    idx, wts, _ = run(hidden_states, weight)
    return idx, wts
